# revision 1
# baseline (speedup 1.0000x reference)
"""Trainium2 Bass kernel for the dual-branch cross-attention module.

Computation (see the module's reference):
    q1,k1,v1 = split(x @ w_qkv1); q2,k2,v2 = split(y @ w_qkv2)   (B,H,L,D)
    a1 = softmax(1 - q1 k2^T / sqrt(D));  xo = a1 @ v1
    a2 = softmax(1 - q2 k1^T / sqrt(D));  yo = a2 @ v2
    out = (xo @ w_p1 + b_p1, yo @ w_p2 + b_p2)

Sharding: batch*heads across 8 cores. Core c handles batch b=c//2 and the
8-head slice h0=(c%2)*8; the host sums the two partial projections per
batch and adds the bias (softmax(1-z) == softmax(-z): shift dropped).

v3: the TRN2 PE p-state ramps to full clock only after ~3us of gap-free
execution, so the whole kernel is emitted as one continuous PE stream:
attention runs as a lag-2 software pipeline (scores run LAG chunks ahead
of the PV matmuls so the scalar-engine exp is never on the PE's critical
path), and the QKV / output-projection matmul groups are interleaved as
contiguous filler blocks between attention chunks. Host pre-casts to
bf16 and pre-transposes all inputs (no on-chip transposes or casts).
PSUM: score-pair tiles [128,1024] (2 bufs, 4 banks) shared with filler
groups + 4 PV accumulators (4 banks). psum->sbuf copies for q/k/v go to
the otherwise-idle gpsimd engine.

Self-contained: shapes/sharding hardcoded; imports only the system bass stack.
"""

import os
import sys
from contextlib import ExitStack

import numpy as np
import ml_dtypes

for _p in ("/opt/trn_rl_repo", os.path.expanduser("~/.axon_site/_ro/trn_rl_repo")):
    if os.path.isdir(_p) and _p not in sys.path:
        sys.path.insert(0, _p)

import concourse.tile as tile
from concourse import bacc, mybir
from concourse.bass_utils import run_bass_kernel_spmd

F32 = mybir.dt.float32
BF16 = mybir.dt.bfloat16
EXP = mybir.ActivationFunctionType.Exp

L = 1024          # sequence length
DIM = 1024        # model dim
D = 64            # head dim
SCALE = D ** -0.5
PROJ = 256        # projection out dim
NCORES = 8
PAIRS = 4         # head pairs per core (8 heads / 2)
KC = 8            # contraction chunks of 128 over DIM
MC = 8            # key-position chunks of 128 over L
LWIN = 512        # query-window (matmul free dim)
NLW = L // LWIN
LAG = 2           # attention pipeline: pv trails scores by LAG chunks

W_NAMES = ("wq1", "wk1", "wv1", "wq2", "wk2", "wv2")


def _build_body(nc, tc, ins, outs, ctx):
    big = ctx.enter_context(tc.tile_pool(name="big", bufs=1))
    ep = ctx.enter_context(tc.tile_pool(name="ep", bufs=4))
    nrm = ctx.enter_context(tc.tile_pool(name="nrm", bufs=4))
    outp = ctx.enter_context(tc.tile_pool(name="outp", bufs=3))
    st_ps = ctx.enter_context(tc.tile_pool(name="st_ps", bufs=2, space="PSUM"))
    pv_ps = ctx.enter_context(tc.tile_pool(name="pv_ps", bufs=4, space="PSUM"))

    # ---- persistent SBUF tiles; DMA straight into final bf16 layouts ----
    xT = big.tile([128, KC, L], BF16, tag="xT")
    yT = big.tile([128, KC, L], BF16, tag="yT")
    w_bf = {nm: big.tile([128, KC, 512], BF16, tag=nm, name=nm)
            for nm in W_NAMES}
    wp_bf = {nm: big.tile([128, PAIRS, PROJ], BF16, tag=nm, name=nm)
             for nm in ("wp1", "wp2")}

    # load order: branch-0 path first so attention can start early
    nc.sync.dma_start(out=w_bf["wv1"], in_=ins["wv1"])
    nc.sync.dma_start(out=xT, in_=ins["xT"])
    nc.sync.dma_start(out=w_bf["wq1"], in_=ins["wq1"])
    nc.sync.dma_start(out=w_bf["wk2"], in_=ins["wk2"])
    nc.sync.dma_start(out=yT, in_=ins["yT"])
    for nm in ("wq2", "wk1", "wv2"):
        nc.sync.dma_start(out=w_bf[nm], in_=ins[nm])
    nc.sync.dma_start(out=wp_bf["wp1"], in_=ins["wp1"])
    nc.sync.dma_start(out=wp_bf["wp2"], in_=ins["wp2"])

    qk = {}     # (name, pair) -> [128, L] bf16 (chan-major q^T / k^T)
    vaug = {}   # (pair, branch) -> [128, MC, 130] bf16, ones at cols 64/129
    onorm = {}  # (pair, branch) -> [128, L] bf16 normalized O^T

    for br in range(2):
        for p in range(PAIRS):
            va = big.tile([128, MC, 130], BF16, tag=f"va_{p}_{br}",
                          name=f"va_{p}_{br}")
            nc.vector.memset(va[:, :, 64:65], 1.0)
            nc.vector.memset(va[:, :, 129:130], 1.0)
            vaug[(p, br)] = va
            onorm[(p, br)] = big.tile([128, L], BF16, tag=f"on_{p}_{br}",
                                      name=f"on_{p}_{br}")
    for nm in ("q1", "k2", "q2", "k1"):
        for p in range(PAIRS):
            qk[(nm, p)] = big.tile([128, L], BF16, tag=f"{nm}_{p}",
                                   name=f"{nm}_{p}")

    # ---- filler groups: contiguous blocks sharing the "st" psum tag ----
    def emit_qk_group(nm, p):
        # q/k tensor-pair: [128, 1024] = 8 accumulating matmuls of N=1024
        dst, wt = qk[(nm, p)], w_bf["w" + nm]
        src = xT if nm in ("q1", "k1") else yT
        mm = st_ps.tile([128, 1024], F32, tag="st", name="st")
        for lw in range(NLW):
            lsl = slice(lw * LWIN, (lw + 1) * LWIN)
            for c in range(KC):
                nc.tensor.matmul(mm[:, lsl], wt[:, c, p * 128:(p + 1) * 128],
                                 src[:, c, lsl],
                                 start=(c == 0), stop=(c == KC - 1))
        nc.vector.tensor_copy(out=dst[:, 0:512], in_=mm[:, 0:512])
        nc.vector.tensor_copy(out=dst[:, 512:1024], in_=mm[:, 512:1024])

    def emit_v_group(br, lt):
        wt = w_bf["wv1" if br == 0 else "wv2"]
        src = xT if br == 0 else yT
        mm = st_ps.tile([128, 1024], F32, tag="st", name="st")
        for c in range(KC):
            nc.tensor.matmul(mm[:, 0:512], src[:, c, lt * 128:(lt + 1) * 128],
                             wt[:, c, :], start=(c == 0), stop=(c == KC - 1))
        for p in range(PAIRS):
            va = vaug[(p, br)]
            nc.vector.tensor_copy(out=va[:, lt, 0:64],
                                  in_=mm[:, p * 128:p * 128 + 64])
            nc.vector.tensor_copy(out=va[:, lt, 65:129],
                                  in_=mm[:, p * 128 + 64:(p + 1) * 128])

    def emit_proj_group(br, lt):
        wt = wp_bf[("wp1", "wp2")[br]]
        out_r = outs[("p1", "p2")[br]].rearrange("(i p) n -> p i n", p=128)
        tsl = slice(lt * 128, (lt + 1) * 128)
        mm = st_ps.tile([128, 1024], F32, tag="st", name="st")
        for pp in range(PAIRS):
            nc.tensor.matmul(mm[:, 0:PROJ], onorm[(pp, br)][:, tsl],
                             wt[:, pp, :], start=(pp == 0),
                             stop=(pp == PAIRS - 1))
        ob = outp.tile([128, PROJ], F32, tag="ob", name="ob")
        nc.vector.tensor_copy(out=ob, in_=mm[:, 0:PROJ])
        nc.sync.dma_start(out=out_r[:, lt, :], in_=ob)

    # ---- attention pipeline pieces ----
    def emit_score(br, p, lw, mc):
        qT = qk[("q1", p)] if br == 0 else qk[("q2", p)]
        kT = qk[("k2", p)] if br == 0 else qk[("k1", p)]
        msl = slice(mc * 128, (mc + 1) * 128)
        lsl = slice(lw * LWIN, (lw + 1) * LWIN)
        st = st_ps.tile([128, 1024], F32, tag="st", name="st")
        nc.tensor.matmul(st[:, 0:512], kT[0:64, msl], qT[0:64, lsl],
                         start=True, stop=True)
        nc.tensor.matmul(st[:, 512:1024], kT[64:128, msl], qT[64:128, lsl],
                         start=True, stop=True)
        e = ep.tile([128, 1024], BF16, tag="E", name="E")
        nc.scalar.activation(out=e, in_=st, func=EXP, scale=-SCALE)
        return e

    def normalize(p, br, lw, head, pv):
        on = onorm[(p, br)]
        lsl = slice(lw * LWIN, (lw + 1) * LWIN)
        ssum = nrm.tile([1, 512], F32, tag="ssum", name="ssum")
        nc.vector.tensor_copy(out=ssum, in_=pv[64:65, :])
        pvo = nrm.tile([64, 512], F32, tag="pvo", name="pvo")
        nc.vector.tensor_copy(out=pvo, in_=pv[0:64, :])
        sb = nrm.tile([64, 512], F32, tag="sb", name="sb")
        nc.gpsimd.partition_broadcast(sb, ssum)
        rb = nrm.tile([64, 512], F32, tag="rb", name="rb")
        nc.vector.reciprocal_approx_fast(out=rb, in_=sb)
        nc.vector.tensor_mul(out=on[head * 64:(head + 1) * 64, lsl],
                             in0=pvo, in1=rb)

    def emit_pv(br, p, lw, mc, e, pvAB):
        va = vaug[(p, br)]
        pvA, pvB = pvAB
        nc.tensor.matmul(pvA, va[:, mc, 0:65], e[:, 0:512],
                         start=(mc == 0), stop=(mc == MC - 1))
        nc.tensor.matmul(pvB, va[:, mc, 65:130], e[:, 512:1024],
                         start=(mc == 0), stop=(mc == MC - 1))
        if mc == MC - 1:
            normalize(p, br, lw, 0, pvA)
            normalize(p, br, lw, 1, pvB)

    # ---- stage A: minimum prerequisites for attention br0/p0 ----
    for lt in range(MC):
        emit_v_group(0, lt)
    for nm in ("q1", "k2"):
        emit_qk_group(nm, 0)

    if os.environ.get("BASSDBG"):
        for nm_, key in (("dbg_q1", ("q1", 0)), ("dbg_k2", ("k2", 0))):
            t = outp.tile([128, L], F32, tag=nm_, name=nm_, bufs=1)
            nc.vector.tensor_copy(out=t[:, 0:512], in_=qk[key][:, 0:512])
            nc.vector.tensor_copy(out=t[:, 512:1024], in_=qk[key][:, 512:1024])
            nc.sync.dma_start(out=outs[nm_], in_=t)
        tva = outp.tile([128, MC * 130], F32, tag="dbg_va", name="dbg_va", bufs=1)
        va0 = vaug[(0, 0)]
        for lt_ in range(MC):
            nc.vector.tensor_copy(out=tva[:, lt_ * 130:(lt_ + 1) * 130],
                                  in_=va0[:, lt_, :])
        nc.sync.dma_start(out=outs["dbg_va"], in_=tva)

    # ---- filler schedule: (due_slot, group_fn) ----
    # br0 slots 0..63 host: q1/k2 for p1-p3 (due before that pair starts),
    # v2 + q2/k1 p0 (due before br1). br1 slots 64..127 host q2/k1 p1-p3
    # and the br0 output projection.
    fillers = []
    for p in (1, 2, 3):
        fillers.append((16 * p - 14, lambda nm="q1", p=p: emit_qk_group(nm, p)))
        fillers.append((16 * p - 10, lambda nm="k2", p=p: emit_qk_group(nm, p)))
    for lt in range(MC):
        fillers.append((22 + 3 * lt, lambda lt=lt: emit_v_group(1, lt)))
    fillers.append((48, lambda: emit_qk_group("q2", 0)))
    fillers.append((52, lambda: emit_qk_group("k1", 0)))
    for p in (1, 2, 3):
        fillers.append((64 + 16 * p - 14,
                        lambda nm="q2", p=p: emit_qk_group(nm, p)))
        fillers.append((64 + 16 * p - 10,
                        lambda nm="k1", p=p: emit_qk_group(nm, p)))
    for lt in range(L // 128):
        fillers.append((70 + 6 * lt, lambda lt=lt: emit_proj_group(0, lt)))
    fillers.sort(key=lambda t: t[0])
    fidx = [0]

    def pump(slot):
        while fidx[0] < len(fillers) and fillers[fidx[0]][0] <= slot:
            fillers[fidx[0]][1]()
            fidx[0] += 1

    # ---- main pipeline over attention chunk slots ----
    slots = [(br, p, lw, mc)
             for br in range(2) for p in range(PAIRS)
             for lw in range(NLW) for mc in range(MC)]
    e_store = {}
    pv_tiles = {}
    for i, (br, p, lw, mc) in enumerate(slots):
        e_store[i] = emit_score(br, p, lw, mc)
        if i == 0 and os.environ.get("BASSDBG"):
            te = outp.tile([128, 1024], F32, tag="dbg_e", name="dbg_e", bufs=1)
            nc.vector.tensor_copy(out=te, in_=e_store[i])
            nc.sync.dma_start(out=outs["dbg_e"], in_=te)
        pump(i)
        j = i - LAG
        if j >= 0:
            brj, pj, lwj, mcj = slots[j]
            if mcj == 0:
                pv_tiles[(brj, pj, lwj)] = (
                    pv_ps.tile([65, 512], F32, tag="pv", name="pv"),
                    pv_ps.tile([65, 512], F32, tag="pv", name="pv"))
            emit_pv(brj, pj, lwj, mcj, e_store.pop(j),
                    pv_tiles[(brj, pj, lwj)])
    for j in (len(slots) - LAG, len(slots) - 1):
        brj, pj, lwj, mcj = slots[j]
        emit_pv(brj, pj, lwj, mcj, e_store.pop(j), pv_tiles[(brj, pj, lwj)])
    pump(10 ** 9)

    if os.environ.get("BASSDBG"):
        for nm_, key in (("dbg_on00", (0, 0)), ("dbg_on31", (3, 1))):
            t = outp.tile([128, L], F32, tag=nm_, name=nm_, bufs=1)
            nc.vector.tensor_copy(out=t[:, 0:512], in_=onorm[key][:, 0:512])
            nc.vector.tensor_copy(out=t[:, 512:1024], in_=onorm[key][:, 512:1024])
            nc.sync.dma_start(out=outs[nm_], in_=t)

    # ---- tail: branch-1 output projection ----
    for lt in range(L // 128):
        emit_proj_group(1, lt)


def build():
    nc = bacc.Bacc("TRN2", target_bir_lowering=False, debug=False,
                   num_devices=NCORES)
    ins = {}
    for nm in ("xT", "yT"):
        ins[nm] = nc.dram_tensor(nm, [128, KC, L], BF16,
                                 kind="ExternalInput").ap()
    for nm in W_NAMES:
        ins[nm] = nc.dram_tensor(nm, [128, KC, 512], BF16,
                                 kind="ExternalInput").ap()
    for nm in ("wp1", "wp2"):
        ins[nm] = nc.dram_tensor(nm, [128, PAIRS, PROJ], BF16,
                                 kind="ExternalInput").ap()
    outs = {}
    for nm in ("p1", "p2"):
        outs[nm] = nc.dram_tensor(nm, [L, PROJ], F32, kind="ExternalOutput").ap()
    if os.environ.get("BASSDBG"):
        outs["dbg_q1"] = nc.dram_tensor("dbg_q1", [128, L], F32,
                                        kind="ExternalOutput").ap()
        outs["dbg_k2"] = nc.dram_tensor("dbg_k2", [128, L], F32,
                                        kind="ExternalOutput").ap()
        outs["dbg_va"] = nc.dram_tensor("dbg_va", [128, MC * 130], F32,
                                        kind="ExternalOutput").ap()
        outs["dbg_e"] = nc.dram_tensor("dbg_e", [128, 1024], F32,
                                       kind="ExternalOutput").ap()
        outs["dbg_pv"] = nc.dram_tensor("dbg_pv", [65, 512], F32,
                                        kind="ExternalOutput").ap()
        outs["dbg_on00"] = nc.dram_tensor("dbg_on00", [128, L], F32,
                                          kind="ExternalOutput").ap()
        outs["dbg_on31"] = nc.dram_tensor("dbg_on31", [128, L], F32,
                                          kind="ExternalOutput").ap()
    with tile.TileContext(nc) as tc:
        with ExitStack() as ctx:
            _build_body(nc, tc, ins, outs, ctx)
    nc.compile()
    return nc


_NC_CACHE = None


def _get_nc():
    global _NC_CACHE
    if _NC_CACHE is None:
        _NC_CACHE = build()
    return _NC_CACHE


def _to_cmaj(a):
    """[DIM, N] fp32 -> [128, DIM//128, N] bf16 (contraction chan-major)."""
    n = a.shape[1]
    return np.ascontiguousarray(
        a.reshape(KC, 128, n).transpose(1, 0, 2)).astype(ml_dtypes.bfloat16)


def make_in_maps(x, y, w_qkv1, w_qkv2, w_p1, w_p2):
    """Shard the full inputs: core c -> batch c//2, head-slice (c%2)*8."""
    in_maps = []
    for c in range(NCORES):
        b, half = divmod(c, 2)
        c0 = half * 512  # channel offset of this core's 8 heads
        m = {
            "xT": _to_cmaj(np.asarray(x[b]).T),
            "yT": _to_cmaj(np.asarray(y[b]).T),
            "wp1": np.ascontiguousarray(
                np.ascontiguousarray(w_p1[c0:c0 + 512, :])
                .reshape(PAIRS, 128, PROJ).transpose(1, 0, 2))
                .astype(ml_dtypes.bfloat16),
            "wp2": np.ascontiguousarray(
                np.ascontiguousarray(w_p2[c0:c0 + 512, :])
                .reshape(PAIRS, 128, PROJ).transpose(1, 0, 2))
                .astype(ml_dtypes.bfloat16),
        }
        for wsrc, names in ((w_qkv1, ("wq1", "wk1", "wv1")),
                            (w_qkv2, ("wq2", "wk2", "wv2"))):
            for j, nm in enumerate(names):
                base = j * DIM + c0
                m[nm] = _to_cmaj(np.ascontiguousarray(wsrc[:, base:base + 512]))
        in_maps.append(m)
    return in_maps


def run_cores(in_maps, trace=False, trace_cores=None):
    nc = _get_nc()
    return run_bass_kernel_spmd(nc, in_maps, list(range(NCORES)),
                                trace=trace, trace_cores=trace_cores)


def kernel(x, y, w_qkv1, w_qkv2, w_p1, b_p1, w_p2, b_p2):
    x = np.asarray(x, dtype=np.float32)
    y = np.asarray(y, dtype=np.float32)
    in_maps = make_in_maps(x, y, np.asarray(w_qkv1), np.asarray(w_qkv2),
                           np.asarray(w_p1), np.asarray(w_p2))
    res = run_cores(in_maps).results
    out1 = np.stack([res[2 * b]["p1"] + res[2 * b + 1]["p1"] for b in range(4)])
    out2 = np.stack([res[2 * b]["p2"] + res[2 * b + 1]["p2"] for b in range(4)])
    out1 += np.asarray(b_p1, dtype=np.float32)
    out2 += np.asarray(b_p2, dtype=np.float32)
    return out1, out2

